# revision 12
# baseline (speedup 1.0000x reference)
"""CombinedDynamicMarginLoss on 8 trn2 NeuronCores.

Strategy: data-parallel over the batch dim N=1024 -> 128 rows per core
(one full SBUF partition tile), each core sees all C=93431 classes so
every per-row reduction is core-local (no collectives).

Device per core (streaming, single pass over the 47.8MB shard):
  - out = 64 * x           (full [128, C] output, ACT engine)
  - rowmax = max_j g(x_ij) (g(x) = x * (x <= 0.4), DVE)
Host glue (1024 rows, negligible):
  - cos_y gather, exclusion of the label column from the max,
    arccos/cos margin math, scatter of final_phi*64 into the output.

The device max includes the label column j=y with the filter applied
(g(cos_y)); since all g values are >= 0 and C is large,
max_other = rowmax exactly whenever g(cos_y) < rowmax. The rare
ambiguous rows (g(cos_y) == rowmax) are recomputed exactly on host.
"""

import numpy as np

import concourse.bacc as bacc
import concourse.mybir as mybir
import concourse.tile as tile
from concourse.bass_utils import run_bass_kernel_spmd

N, C = 1024, 93431
NCORES = 8
R = N // NCORES  # 128 rows per core

S = 64.0
M1 = 1.0
M2 = 0.5
M3 = 0.0
ALPHA = 0.1
THRESH = 0.4
NEG_BIG = -1.0e9

T = 4096                      # column tile width
NT = (C + T - 1) // T         # 23 tiles (22 full + 3319 remainder)

_CACHE: dict = {}
LAST_RESULT = None            # BassKernelResults of the last run (for test.py)
RUN_KWARGS: dict = {}         # test.py can set {"trace": True}


def _build():
    f32 = mybir.dt.float32
    # Bacc (not raw Bass): its compile pass splits multi-wait sync onto
    # separate event-semaphore instructions — DMACopy only encodes 1 wait.
    nc = bacc.Bacc(None)
    x = nc.declare_dram_parameter("x", [R, C], f32, isOutput=False)
    y = nc.declare_dram_parameter("y", [R, C], f32, isOutput=True)
    mx = nc.declare_dram_parameter("mx", [R, 1], f32, isOutput=True)

    # 0.4 * 64 is exact in fp32 (power-of-two scale), so filtering the
    # scaled tensor (yt <= 25.6) * yt equals 64 * g(x) bit-exactly.
    thresh_s = float(np.float32(THRESH) * np.float32(S))

    # Loads on the sync HWDGE ring, stores on the scalar engine's HWDGE
    # ring (same-engine ordering after the mul that produced the data).
    with tile.TileContext(nc) as tc:
        with (
            tc.tile_pool(name="xin", bufs=4) as xpool,
            tc.tile_pool(name="yout", bufs=4) as ypool,
            tc.tile_pool(name="gtmp", bufs=2) as gpool,
            tc.tile_pool(name="stat", bufs=1) as statpool,
        ):
            maxbuf = statpool.tile([R, NT], f32)
            for t in range(NT):
                w = min(T, C - t * T)
                xt = xpool.tile([R, T], f32, tag="xt")
                nc.sync.dma_start(out=xt[:, :w], in_=x[:, t * T : t * T + w])

                yt = ypool.tile([R, T], f32, tag="yt")
                nc.scalar.mul(yt[:, :w], xt[:, :w], S)
                nc.scalar.dma_start(out=y[:, t * T : t * T + w], in_=yt[:, :w])

                # g64 = (yt <= 25.6) * yt == 64 * g(x), one DVE op
                g = gpool.tile([R, T], f32, tag="g")
                nc.vector.scalar_tensor_tensor(
                    out=g[:, :w],
                    in0=yt[:, :w],
                    scalar=thresh_s,
                    in1=yt[:, :w],
                    op0=mybir.AluOpType.is_le,
                    op1=mybir.AluOpType.mult,
                )
                nc.vector.tensor_reduce(
                    out=maxbuf[:, t : t + 1],
                    in_=g[:, :w],
                    axis=mybir.AxisListType.X,
                    op=mybir.AluOpType.max,
                )

            mfin = statpool.tile([R, 1], f32)
            nc.vector.tensor_reduce(
                out=mfin[:],
                in_=maxbuf[:],
                axis=mybir.AxisListType.X,
                op=mybir.AluOpType.max,
            )
            nc.scalar.dma_start(out=mx[:], in_=mfin[:])
    # run_bass_via_pjrt serializes the module at jit-lowering time without
    # finalizing; Bacc's register allocation happens in finalize().
    nc.finalize()
    return nc


def _get_nc():
    if "nc" not in _CACHE:
        _CACHE["nc"] = _build()
    return _CACHE["nc"]


def kernel(logits, labels):
    global LAST_RESULT
    logits = np.ascontiguousarray(np.asarray(logits, dtype=np.float32))
    labels = np.asarray(labels).astype(np.int64)
    assert logits.shape == (N, C)

    nc = _get_nc()
    in_maps = [{"x": logits[k * R : (k + 1) * R]} for k in range(NCORES)]
    res = run_bass_kernel_spmd(nc, in_maps, list(range(NCORES)), **RUN_KWARGS)
    LAST_RESULT = res

    out = np.concatenate([res.results[k]["y"] for k in range(NCORES)], axis=0)
    M64 = np.concatenate([res.results[k]["mx"] for k in range(NCORES)], axis=0)[:, 0]
    M = (M64 * np.float32(1.0 / S)).astype(np.float32)  # exact (power of two)

    # ---- host glue: per-row scalars (N=1024) ----
    valid = labels != -1
    lab = np.where(valid, labels, 0)
    rows = np.arange(N)
    cos_y = logits[rows, lab]                                   # f32
    g_cos = np.where(cos_y <= THRESH, cos_y, 0.0).astype(np.float32)

    max_other = M.copy()
    # ambiguous: the device max may have been achieved at the label column
    amb = np.nonzero((g_cos >= M) & valid)[0]
    for i in amb:
        g = np.where(logits[i] <= THRESH, logits[i], 0.0).astype(np.float32)
        g[lab[i]] = NEG_BIG
        max_other[i] = g.max()

    h = (np.float32(1.0) - (cos_y - max_other)).astype(np.float32)
    m_i = (np.float32(M2) + np.float32(ALPHA) * h).astype(np.float32)
    theta = np.arccos(np.clip(cos_y, -1.0, 1.0)).astype(np.float32)
    phi = (np.cos(np.float32(M1) * theta + m_i) - np.float32(M3)).astype(np.float32)
    final_phi = np.where(phi < cos_y, phi, cos_y).astype(np.float32)

    out[rows[valid], lab[valid]] = final_phi[valid] * np.float32(S)
    return out
